# revision 53
# baseline (speedup 1.0000x reference)
"""TRN2 Bass/Tile kernel for nn_Block_19756849561899 (pre-LN transformer
block: LN -> MHA -> residual -> LN -> MLP(gelu) -> residual).

Self-contained: kernel(**inputs) takes the full fp32 tensors, shards work
across 8 NeuronCores (one batch per core-pair; each core owns half the
sequence as queries and redundantly builds K/V for its batch), compiles a
Bass/Tile program once per process, runs it SPMD, and reassembles the full
output.

Schedule (vs the original per-head-pair baseline): j-major attention whose
ScalarE stream is pure exp (the pacer, ~1.15us per 1024-wide chunk); LN1 /
K / V / Q builds and the first half's proj/LN2/fc1 ride as PE fillers
inside the head streams; softmax normalization is deferred two heads (the
Z DRAM-broadcast roundtrip never blocks the DVE queue); "ballast" matmuls
pad PE-idle gaps so the HAM clock gate stays at 2.4GHz; LN stats use
bn_stats; the V bias folds into the proj bias (softmax rows sum to 1); the
fc1 bias folds into the DVE evac so gelu runs as a few wide pure ACTs; fc2
runs in fp8 DoubleRow with x16-scaled weights.
"""

import contextlib

import numpy as np
import ml_dtypes

import concourse.bass as bass
import concourse.mybir as mybir
import concourse.tile as tile
from concourse.masks import make_identity

fp32 = mybir.dt.float32
bf16 = mybir.dt.bfloat16
fp8 = mybir.dt.float8e4
AF = mybir.ActivationFunctionType
ALU = mybir.AluOpType
AX = mybir.AxisListType

C = 384
CS = 3          # C / 128
H = 6
HP = 3          # head pairs
DH = 64
HID = 1536
KS = 12         # HID / 128
VW = 72         # padded V row width (DoubleRow needs 16B-aligned pair stride)
W2SCALE = 16.0  # fp8 fc2 weight scaling (avoids e4m3 denormals)
EPS = 1e-6
NBIAS = 24
SCHRA_A = float(2 ** 23 / np.log(2))   # Schraudolph fast-exp scale
SCHRA_B = 1064986823.0                 # Schraudolph bias (max-err-optimal)
DVE_EXP_CHUNKS = ()                # exp chunks offloaded ScalarE -> DVE


def build(nc, SEQ=2048, act_fn=AF.Gelu):
    TT = SEQ // 128          # token tiles over full sequence
    QTT = TT // 2            # token tiles in own (query) half
    QLEN = SEQ // 2
    QF = min(512, QLEN)      # q free-dim tile
    NJ = QLEN // QF
    NF = min(512, SEQ)       # seq free-dim tile for K^T build
    NN = SEQ // NF
    NB = QF // 128           # token blocks per q-tile
    CK = 2                   # key tiles per S/exp chunk
    NCH = TT // CK           # chunks per (h, j)
    chunks = [(k0, min(CK, TT - k0)) for k0 in range(0, TT, CK)]

    xin = nc.dram_tensor("xin", [SEQ, C], fp32, kind="ExternalInput")
    wqk_d = nc.dram_tensor("wqk", [128, CS, 768], bf16, kind="ExternalInput")
    wv_d = nc.dram_tensor("wv", [128, CS, C], bf16, kind="ExternalInput")
    wp_d = nc.dram_tensor("wp", [128, CS, C], bf16, kind="ExternalInput")
    wf1_d = nc.dram_tensor("wf1", [128, CS, HID], bf16, kind="ExternalInput")
    wf2_d = nc.dram_tensor("wf2", [128, KS // 2, 2, C], fp8, kind="ExternalInput")
    bias_d = nc.dram_tensor("bias", [128, NBIAS], fp32, kind="ExternalInput")
    yout = nc.dram_tensor("yout", [QLEN, C], fp32, kind="ExternalOutput")

    xin_t = xin.ap().rearrange("(t p) c -> p t c", p=128)     # [128, TT, C]
    yout_t = yout.ap().rearrange("(t p) c -> p t c", p=128)   # [128, QTT, C]

    with tile.TileContext(nc) as tc, contextlib.ExitStack() as ctx:
        per = ctx.enter_context(tc.tile_pool(name="per", bufs=1))
        dr = ctx.enter_context(tc.tile_pool(name="dr", bufs=2, space="DRAM"))
        ldx = ctx.enter_context(tc.tile_pool(name="ldx", bufs=6))
        xnp = ctx.enter_context(tc.tile_pool(name="xnp", bufs=6))
        expp = ctx.enter_context(tc.tile_pool(name="expp", bufs=6))
        rzp = ctx.enter_context(tc.tile_pool(name="rzp", bufs=3))
        ytp = ctx.enter_context(tc.tile_pool(name="ytp", bufs=4))
        sta = ctx.enter_context(tc.tile_pool(name="sta", bufs=1))
        # PSUM: 4 + 2 + 2 banks
        pss = ctx.enter_context(tc.tile_pool(name="pss", bufs=2, space="PSUM"))
        psa = ctx.enter_context(tc.tile_pool(name="psa", bufs=2, space="PSUM"))
        psm = ctx.enter_context(tc.tile_pool(name="psm", bufs=2, space="PSUM"))

        x_own = per.tile([128, QTT, C], fp32)
        ldx_tiles = {}
        for t in range(4):
            nc.sync.dma_start(x_own[:, t, :], xin_t[:, t, :])
        wqk = per.tile([128, CS, 768], bf16)
        nc.sync.dma_start(wqk[:], wqk_d.ap())
        bias = per.tile([128, NBIAS], fp32)
        nc.sync.dma_start(bias[:], bias_d.ap())
        for t in range(4, QTT):
            nc.sync.dma_start(x_own[:, t, :], xin_t[:, t, :])
        wv = per.tile([128, CS, C], bf16)
        nc.sync.dma_start(wv[:], wv_d.ap())
        for t in range(QTT, QTT + 6):
            xts = ldx.tile([128, C], fp32, tag="xt")
            ldx_tiles[t] = xts
            nc.sync.dma_start(xts[:], xin_t[:, t, :])
        wp = per.tile([128, CS, C], bf16)
        wf1 = per.tile([128, CS, HID], bf16)
        wf2 = per.tile([128, KS // 2, 2, C], fp8)
        ident = per.tile([128, 128], bf16)
        make_identity(nc, ident)

        x2 = per.tile([128, QTT, C], fp32)
        KT = per.tile([128, HP, SEQ], bf16)
        QT = per.tile([128, HP, QLEN], bf16)
        Vsb = per.tile([128, TT, H, VW], fp8)
        xnT = per.tile([128, CS, SEQ], bf16)
        xn2T = per.tile([128, CS, QLEN], bf16)
        AT = per.tile([128, HP, QLEN], bf16)
        hpre = per.tile([128, NJ, KS, QF], bf16)
        hful = per.tile([128, NJ, KS // 2, 2, QF], fp8)

        nc.vector.memset(Vsb[:, :, :, DH], 1.0)   # Z ones column

        # PE warm-up burst using the identity tile (no DMA dependency), so
        # the HAM clock-gate opens (1.2->2.4GHz) while the x/weight DMAs are
        # still in flight.
        warm = psa.tile([128, NF], fp32, tag="aa", name="warm")
        for _ in range(24):
            nc.tensor.matmul(warm[:, :128], ident[:], ident[:],
                             start=True, stop=True)
        warmsink = per.tile([128, 1], fp32)
        nc.vector.tensor_copy(warmsink[:, 0:1], warm[:, 0:1])

        # stats: col0 mean, col1 var, col2 rstd, col3 lnb(-mean*rstd), 4-5 scr
        stats = sta.tile([128, TT, 6], fp32)
        bnst = sta.tile([128, TT, 6], fp32)

        def ln_stats_tile(xt, t):
            nc.vector.bn_stats(bnst[:, t, :], xt)
            nc.vector.bn_aggr(stats[:, t, 0:2], bnst[:, t, :])

        def ln_group_rstd(sg):
            """batched rstd via DVE Newton on var: sg [128, G, 6]."""
            mean, var = sg[:, :, 0], sg[:, :, 1]
            y, lnb_ = sg[:, :, 2], sg[:, :, 3]
            tmp = sg[:, :, 4]
            nc.vector.tensor_scalar_add(var, var, EPS)
            # y0 = 1 folded into first Newton step: y1 = 1.5 - 0.5*v
            nc.vector.tensor_scalar(
                y, var, -0.5, 1.5, op0=ALU.mult, op1=ALU.add)
            for _ in range(2):
                nc.vector.tensor_tensor(tmp, y, y, ALU.mult)
                nc.vector.tensor_tensor(tmp, tmp, var, ALU.mult)
                nc.vector.tensor_scalar(
                    tmp, tmp, -0.5, 1.5, op0=ALU.mult, op1=ALU.add)
                nc.vector.tensor_tensor(y, y, tmp, ALU.mult)
            nc.vector.tensor_tensor(lnb_, mean, y, ALU.mult)
            nc.vector.tensor_scalar_mul(lnb_, lnb_, -1.0)

        def ln_apply(xt, st, xn_out, on_act=False):
            if on_act:
                nc.scalar.activation(
                    xn_out, xt, AF.Identity, bias=st[:, 3:4], scale=st[:, 2:3])
            else:
                nc.vector.tensor_scalar(
                    xn_out, xt, st[:, 2:3], st[:, 3:4], op0=ALU.mult, op1=ALU.add)

        def transpose_to(xn, dstT, t, on_act=False):
            """3 PE transposes of xn [128, C] bf16 into dstT[:, :, t*128...].

            Allocates from psa (NOT psm) so this can run inside an open
            attention-head PV accumulation, which holds a psm buffer."""
            ptr = psa.tile([128, CS * 128], bf16, tag="aa", name="ptrA")
            for cs in range(CS):
                nc.tensor.transpose(
                    ptr[:, cs * 128:(cs + 1) * 128],
                    xn[:, cs * 128:(cs + 1) * 128], ident[:])
            src = ptr[:, :CS * 128].rearrange("p (cs n) -> p cs n", n=128)
            dst = dstT[:, :, t * 128:(t + 1) * 128]
            if on_act:
                nc.scalar.copy(dst, src)
            else:
                nc.vector.tensor_copy(dst, src)

        v4 = Vsb.rearrange("p t h e -> p t (h e)")

        def ln1_group(g0, G, on_act=False, v_act=False):
            """LN1 + transpose + V for tiles [g0, g0+G)."""
            xtiles = {}
            for t in range(g0, g0 + G):
                if t < QTT:
                    xt = x_own[:, t, :]
                elif t in ldx_tiles:
                    xt = ldx_tiles[t][:]
                else:
                    xts = ldx.tile([128, C], fp32, tag="xt")
                    xt = xts[:]
                    nc.sync.dma_start(xt, xin_t[:, t, :])
                xtiles[t] = xt
                ln_stats_tile(xt, t)
            ln_group_rstd(stats[:, g0:g0 + G, :])
            for t in range(g0, g0 + G):
                xn = xnp.tile([128, C], bf16, tag="xn")
                ln_apply(xtiles[t], stats[:, t, :], xn[:], on_act=on_act)
                transpose_to(xn, xnT, t, on_act=on_act)

                pv = psa.tile([128, NF], fp32, tag="aa", name="pvA")
                for cs in range(CS):
                    nc.tensor.matmul(
                        pv[:, :C], xnT[:, cs, t * 128:(t + 1) * 128],
                        wv[:, cs, :],
                        start=(cs == 0), stop=(cs == CS - 1))
                pv3 = pv[:, :C].rearrange("p (h d) -> p h d", d=DH)
                # scatter V rows into the padded [H, VW] layout (ones col kept)
                vdst = v4[:, t, :].rearrange("p (h e) -> p h e", e=VW)[:, :, :DH]
                if on_act or v_act:
                    nc.scalar.copy(vdst, pv3)
                else:
                    nc.vector.tensor_copy(vdst, pv3)

        def build_k(n, on_act=False):
            """K^T columns for seq tile n (NF tokens)."""
            for m in range(HP, 2 * HP):
                pk = psa.tile([128, NF], fp32, tag="aa", name="pkA")
                for cs in range(CS):
                    nc.tensor.matmul(
                        pk[:, :NF],
                        wqk[:, cs, m * 128:(m + 1) * 128],
                        xnT[:, cs, n * NF:(n + 1) * NF],
                        start=(cs == 0), stop=(cs == CS - 1))
                dst = KT[:, m - HP, n * NF:(n + 1) * NF]
                if on_act:
                    nc.scalar.add(dst, pk[:, :NF], bias[:, m:m + 1])
                else:
                    nc.vector.tensor_scalar_add(dst, pk[:, :NF], bias[:, m:m + 1])

        def build_q(m, j, on_act=False):
            """Q^T columns for head-pair row block m, query tile j."""
            pk = psa.tile([128, NF], fp32, tag="aa", name="pkA")
            for cs in range(CS):
                nc.tensor.matmul(
                    pk[:, :QF],
                    wqk[:, cs, m * 128:(m + 1) * 128],
                    xnT[:, cs, j * QF:(j + 1) * QF],
                    start=(cs == 0), stop=(cs == CS - 1))
            dst = QT[:, m, j * QF:(j + 1) * QF]
            if on_act:
                nc.scalar.add(dst, pk[:, :QF], bias[:, m:m + 1])
            else:
                nc.vector.tensor_scalar_add(dst, pk[:, :QF], bias[:, m:m + 1])

        # ---------------- attention ----------------
        def pv_pair(po, ech, k0, nk, h):
            # fp8 DoubleRow: one matmul contracts a PAIR of key tiles;
            # lhsT [128, 2, 65], rhs [128, 2, QF] -> out [65, QF]
            if nk == CK:
                nc.tensor.matmul(
                    po[:DH + 1, :], Vsb[:, k0:k0 + 2, h, :DH + 1], ech[:, :2, :],
                    start=(k0 == 0), stop=(k0 + 2 == TT),
                    perf_mode=mybir.MatmulPerfMode.DoubleRow)
            else:
                for i in range(nk):
                    kt = k0 + i
                    nc.tensor.matmul(
                        po[:DH + 1, :], Vsb[:, kt, h, :DH + 1], ech[:, i, :],
                        start=(kt == 0), stop=(kt == TT - 1))

        def attention_head(h, j, fillers, norm_prev=None, nballast=0):
            """S -> exp -> PV for head h, query tile j. fillers is a list of
            thunks emitting PE-side work; one is drained after each S chunk.
            norm_prev (the previous head's normalize thunk) is emitted before
            this head's rz-pool allocations so the pool rotation stays safe.
            nballast junk matmuls per chunk (into the S PSUM tile, which the
            real S matmuls then overwrite) keep the HAM activity window busy
            so the PE clock-gate stays at 2.4GHz through ACT-paced stretches."""
            hp, hb = h // 2, (h % 2) * 64
            if norm_prev is not None:
                norm_prev()
            po = psm.tile([128, QF], fp32, tag="sm", name="po")
            echunks = []
            for ci, (k0, nk) in enumerate(chunks):
                psS = pss.tile([128, CK * QF], fp32, tag="ss")
                for _ in range(nballast):
                    nc.tensor.matmul(psS[:, :QF], ident[:], xnT[:, 0, :QF],
                                     start=True, stop=True)
                for i in range(nk):
                    kt = k0 + i
                    nc.tensor.matmul(
                        psS[:, i * QF:(i + 1) * QF],
                        KT[hb:hb + 64, hp, kt * 128:(kt + 1) * 128],
                        QT[hb:hb + 64, hp, j * QF:(j + 1) * QF],
                        start=True, stop=True)
                ech = expp.tile([128, CK, QF], fp8, tag="ech")
                if ci in DVE_EXP_CHUNKS and nk == CK:
                    # Schraudolph bit-trick exp on the DVE, offloading the
                    # ScalarE exp stream (which paces the attention phase):
                    # exp(x) ~= bitcast_f32(int32(x*2^23/ln2 + B)); the bit
                    # error (<3% rel) is below the fp8 quantization already
                    # applied to the exp weights.
                    scr = rzp.tile([128, CK * QF], mybir.dt.int32,
                                   tag="scr", bufs=2)
                    nc.vector.tensor_scalar(
                        scr[:], psS[:, :CK * QF], SCHRA_A, SCHRA_B,
                        op0=ALU.mult, op1=ALU.add)
                    nc.vector.tensor_copy(
                        ech.rearrange("p a b -> p (a b)"),
                        scr[:].bitcast(fp32))
                else:
                    nc.scalar.activation(
                        ech[:, :nk, :], psS[:, :nk * QF], AF.Exp)
                echunks.append((ech, k0, nk))
                if ci > 0:
                    pech, pk0, pnk = echunks[ci - 1]
                    pv_pair(po, pech, pk0, pnk, h)
                if fillers:
                    fillers.pop(0)()
            lech, lk0, lnk = echunks[-1]
            pv_pair(po, lech, lk0, lnk, h)

            # Deferred normalize: evacuate UNNORMALIZED attention + the Z row
            # immediately (frees the po PSUM bank without waiting on the
            # Z-broadcast DMA roundtrip), and hand back a thunk that divides
            # AT by Z in place once the broadcast has landed.
            at_sl = AT[hb:hb + 64, hp, j * QF:(j + 1) * QF]
            at_un = rzp.tile([64, QF], bf16, tag="atun")
            nc.vector.tensor_copy(at_un[:], po[:64, :])
            rz = rzp.tile([128, QF], fp32, tag="rz")
            nc.vector.tensor_copy(rz[64:65, :], po[64:65, :])
            zscr = dr.tile([1, QF], fp32, tag="zscr")
            nc.sync.dma_start(zscr[:], rz[64:65, :])
            rzb = rzp.tile([64, QF], fp32, tag="rzb")
            nc.sync.dma_start(rzb[:], zscr.to_broadcast([64, QF]))
            rzr = rzp.tile([64, QF], fp32, tag="rzr")

            def normalize():
                # emitted one head later, when the Z broadcast DMA has
                # landed, so the DVE queue never stalls waiting for it
                nc.vector.reciprocal_approx_fast(out=rzr[:], in_=rzb[:])
                nc.vector.tensor_tensor(at_sl, at_un[:], rzr[:], ALU.mult)
            return normalize

        def transpose_add(src_sb, dst, res):
            # src_sb [128, NB*128] bf16 -> transpose -> dst = res + src^T
            ptr = psa.tile([128, max(QF, CS * 128)], bf16, tag="aa", name="ptrC")
            for b in range(NB):
                nc.tensor.transpose(
                    ptr[:, b * 128:(b + 1) * 128],
                    src_sb[:, b * 128:(b + 1) * 128], ident[:])
            nc.vector.tensor_tensor(
                dst, ptr[:, :NB * 128].rearrange("p (b n) -> p b n", n=128),
                res, ALU.add)

        def proj_m(j, m, on_act=False, pool=None):
            t0 = j * NB
            if True:
                p_ = pool or psa
                pp = p_.tile([128, NF], fp32,
                             tag=("sm" if p_ is psm else "aa"), name="pp")
                for hp in range(HP):
                    nc.tensor.matmul(
                        pp[:, :QF], wp[:, hp, m * 128:(m + 1) * 128],
                        AT[:, hp, j * QF:(j + 1) * QF],
                        start=(hp == 0), stop=(hp == HP - 1))
                y1T = ytp.tile([128, QF], bf16, tag="yT")
                if on_act:
                    nc.scalar.add(y1T[:], pp[:, :QF], bias[:, 6 + m:7 + m])
                else:
                    nc.vector.tensor_scalar_add(
                        y1T[:], pp[:, :QF], bias[:, 6 + m:7 + m])
                transpose_add(
                    y1T,
                    x2[:, t0:t0 + NB, m * 128:(m + 1) * 128],
                    x_own[:, t0:t0 + NB, m * 128:(m + 1) * 128])

        def proj_j(j, on_act=False):
            for m in range(CS):
                proj_m(j, m, on_act=on_act)

        def ln2_half(j, half):
            t0 = j * NB + 2 * half
            for t in range(t0, t0 + 2):
                ln_stats_tile(x2[:, t, :], t)
            ln_group_rstd(stats[:, t0:t0 + 2, :])
            for t in range(t0, t0 + 2):
                xn2 = xnp.tile([128, C], bf16, tag="xn")
                ln_apply(x2[:, t, :], stats[:, t, :], xn2[:])
                transpose_to(xn2, xn2T, t)

        def ln2_j(j):
            ln2_half(j, 0)
            ln2_half(j, 1)

        def fc1_group(j, ks, on_act=False, pool=None):
            """fc1 matmuls for hidden block ks, query tile j; DVE evac adds
            the fc1 bias so the gelu can be one big pure ACT later."""
            p_ = pool or psa
            pf1 = p_.tile([128, NF], fp32,
                          tag=("sm" if p_ is psm else "aa"), name="pf1")
            for cs in range(CS):
                nc.tensor.matmul(
                    pf1[:, :QF], wf1[:, cs, ks * 128:(ks + 1) * 128],
                    xn2T[:, cs, j * QF:(j + 1) * QF],
                    start=(cs == 0), stop=(cs == CS - 1))
            if on_act:
                nc.scalar.add(
                    hpre[:, j, ks, :], pf1[:, :QF], bias[:, 9 + ks:10 + ks])
            else:
                nc.vector.tensor_scalar_add(
                    hpre[:, j, ks, :], pf1[:, :QF], bias[:, 9 + ks:10 + ks])

        def gelu_j(j):
            nc.scalar.activation(
                hful[:, j].rearrange("p g i q -> p (g i q)"),
                hpre[:, j].rearrange("p k q -> p (k q)"), act_fn)

        def gelu_part(j, g0, ng):
            nc.scalar.activation(
                hful[:, j, g0:g0 + ng].rearrange("p g i q -> p (g i q)"),
                hpre[:, j, 2 * g0:2 * (g0 + ng), :]
                    .rearrange("p k q -> p (k q)"), act_fn)

        def fc2_j(j, alt=False):
            t0 = j * NB
            for m in range(CS):
                pool = psm if (alt and m % 2) else psa
                pf2 = pool.tile([128, NF], fp32,
                                tag=("sm" if pool is psm else "aa"), name="pf2")
                for g in range(KS // 2):
                    nc.tensor.matmul(
                        pf2[:, :QF], wf2[:, g, :, m * 128:(m + 1) * 128],
                        hful[:, j, g, :, :],
                        start=(g == 0), stop=(g == KS // 2 - 1),
                        perf_mode=mybir.MatmulPerfMode.DoubleRow)
                y2T = ytp.tile([128, QF], bf16, tag="yT")
                nc.vector.tensor_scalar(
                    y2T[:], pf2[:, :QF], 1.0 / W2SCALE, bias[:, 21 + m:22 + m],
                    op0=ALU.mult, op1=ALU.add)
                if m < CS - 1:
                    transpose_add(
                        y2T,
                        x2[:, t0:t0 + NB, m * 128:(m + 1) * 128],
                        x2[:, t0:t0 + NB, m * 128:(m + 1) * 128])
                else:
                    # final column: per-tile adds so each tile's output DMA
                    # can start the moment that tile is complete
                    ptr = psa.tile([128, NF], bf16, tag="aa", name="ptrC")
                    for b in range(NB):
                        nc.tensor.transpose(
                            ptr[:, b * 128:(b + 1) * 128],
                            y2T[:, b * 128:(b + 1) * 128], ident[:])
                    for b in range(NB):
                        t = t0 + b
                        nc.vector.tensor_tensor(
                            x2[:, t, m * 128:(m + 1) * 128],
                            ptr[:, b * 128:(b + 1) * 128],
                            x2[:, t, m * 128:(m + 1) * 128], ALU.add)
                        nc.sync.dma_start(yout_t[:, t, :], x2[:, t, :])

        # ---------------- emission ----------------
        G = 4

        def tail_ballast(n):
            bt = pss.tile([128, CK * QF], fp32, tag="ss", name="bal")
            for _ in range(n):
                nc.tensor.matmul(bt[:, :QF], ident[:], xnT[:, 0, :QF],
                                 start=True, stop=True)

        # startup: just enough for head 0 to start (LN1 group 0 feeds
        # K columns 0-511 and Q(hp0); everything else rides as fillers)
        ln1_group(0, G, on_act=True)
        tail_ballast(6)    # hold the PE clock gate open through the LN1
        build_k(0, on_act=True)
        tail_ballast(4)    # latency chain (stats -> rstd -> apply -> K/Q)
        build_q(0, 0, on_act=True)
        # deferred weight loads (needed only from proj/MLP onward) so the
        # x-tile loads win the DMA queues at startup
        nc.sync.dma_start(wp[:], wp_d.ap())
        nc.sync.dma_start(wf1[:], wf1_d.ap())
        nc.sync.dma_start(wf2[:], wf2_d.ap())

        # Pending normalize thunks: each head's softmax divide is emitted at
        # the START of the head two slots later, giving the Z-broadcast DMA
        # roundtrip two full head-times to land before the DVE touches it.
        pend = []

        def head(h, j, fillers, nballast=1):
            nprev = pend.pop(0) if len(pend) >= 2 else None
            pend.append(attention_head(h, j, fillers, nprev, nballast))

        # ---- j = 0: remaining LN1 groups / K columns / Q rows ride as
        # fillers inside head 0's chunk stream (S chunk c2k needs K column
        # group k; PV chunk c2k reads V tiles 4k..4k+3).
        fill_h0 = [
            lambda: ln1_group(G, G),
            lambda: build_k(1),
            lambda: ln1_group(2 * G, G),
            lambda: build_k(2),
            lambda: ln1_group(3 * G, G),
            lambda: build_k(3),
            lambda: build_q(1, 0),
            lambda: build_q(2, 0),
        ]
        head(0, 0, fill_h0, nballast=0)
        for h in range(1, H):
            head(h, 0, [])
        build_q(0, 1)
        # ---- j = 1: proj(0) / ln2(0) / fc1(0) ride as fillers so the MLP
        # of the first half overlaps the second half's attention.
        fills1 = [
            [lambda: build_q(1, 1)],
            [lambda: proj_m(0, 0), lambda: proj_m(0, 1), lambda: proj_m(0, 2),
             lambda: build_q(2, 1)],
            [lambda: ln2_half(0, 0), lambda: ln2_half(0, 1)],
            [lambda ks=ks: fc1_group(0, ks) for ks in range(0, 4)],
            [lambda ks=ks: fc1_group(0, ks) for ks in range(4, 8)],
            [lambda ks=ks: fc1_group(0, ks) for ks in range(8, 12)],
        ]
        for h in range(H):
            head(h, 1, fills1[h])
        # ---- tail ----
        pend.pop(0)()      # normalize h4/j1
        pend.pop(0)()      # normalize h5/j1
        gelu_part(0, 0, 3)  # ACT: right after the last exp
        gelu_part(0, 3, 3)
        # PE idles ~5us here waiting on the norms + gelu(0); that idle would
        # close the HAM clock gate (1.2GHz for the whole tail) - pad it.
        tail_ballast(18)
        proj_j(1)
        # fc2(0) split accumulation: m0/m1's first three g-groups only need
        # the first gelu(0) half
        pf0s = []
        for m in range(2):
            pf2 = psa.tile([128, NF], fp32, tag="aa", name="pf2")
            pf0s.append(pf2)
            for g in range(3):
                nc.tensor.matmul(
                    pf2[:, :QF], wf2[:, g, :, m * 128:(m + 1) * 128],
                    hful[:, 0, g, :, :], start=(g == 0), stop=False,
                    perf_mode=mybir.MatmulPerfMode.DoubleRow)
        for m in range(CS):
            if m < 2:
                pf2 = pf0s[m]
                for g in range(3, KS // 2):
                    nc.tensor.matmul(
                        pf2[:, :QF], wf2[:, g, :, m * 128:(m + 1) * 128],
                        hful[:, 0, g, :, :], start=False, stop=(g == KS // 2 - 1),
                        perf_mode=mybir.MatmulPerfMode.DoubleRow)
            else:
                pf2 = psa.tile([128, NF], fp32, tag="aa", name="pf2")
                for g in range(KS // 2):
                    nc.tensor.matmul(
                        pf2[:, :QF], wf2[:, g, :, m * 128:(m + 1) * 128],
                        hful[:, 0, g, :, :], start=(g == 0), stop=(g == KS // 2 - 1),
                        perf_mode=mybir.MatmulPerfMode.DoubleRow)
            y2T = ytp.tile([128, QF], bf16, tag="yT")
            nc.vector.tensor_scalar(
                y2T[:], pf2[:, :QF], 1.0 / W2SCALE, bias[:, 21 + m:22 + m],
                op0=ALU.mult, op1=ALU.add)
            if m < CS - 1:
                transpose_add(
                    y2T, x2[:, 0:NB, m * 128:(m + 1) * 128],
                    x2[:, 0:NB, m * 128:(m + 1) * 128])
            else:
                ptr = psa.tile([128, NF], bf16, tag="aa", name="ptrC")
                for b in range(NB):
                    nc.tensor.transpose(
                        ptr[:, b * 128:(b + 1) * 128],
                        y2T[:, b * 128:(b + 1) * 128], ident[:])
                for b in range(NB):
                    nc.vector.tensor_tensor(
                        x2[:, b, m * 128:(m + 1) * 128],
                        ptr[:, b * 128:(b + 1) * 128],
                        x2[:, b, m * 128:(m + 1) * 128], ALU.add)
                    nc.sync.dma_start(yout_t[:, b, :], x2[:, b, :])
        tail_ballast(8)
        ln2_j(1)
        tail_ballast(4)
        for q in range(3):
            for ks in range(4 * q, 4 * q + 4):
                fc1_group(1, ks)
            if q < 2:
                gelu_part(1, 2 * q, 2)
                tail_ballast(5)
        # fc2(1) split accumulation: m0/m1's first four g-groups issue while
        # the last gelu part runs (psa has only 2 bufs, so m2 stays whole)
        pf2s = []
        for m in range(2):
            pf2 = psa.tile([128, NF], fp32, tag="aa", name="pf2")
            pf2s.append(pf2)
            for g in range(4):
                nc.tensor.matmul(
                    pf2[:, :QF], wf2[:, g, :, m * 128:(m + 1) * 128],
                    hful[:, 1, g, :, :], start=(g == 0), stop=False,
                    perf_mode=mybir.MatmulPerfMode.DoubleRow)
        gelu_part(1, 4, 2)
        t0 = NB
        for m in range(CS):
            if m < 2:
                pf2 = pf2s[m]
                for g in (4, 5):
                    nc.tensor.matmul(
                        pf2[:, :QF], wf2[:, g, :, m * 128:(m + 1) * 128],
                        hful[:, 1, g, :, :], start=False, stop=(g == 5),
                        perf_mode=mybir.MatmulPerfMode.DoubleRow)
            else:
                pf2 = psa.tile([128, NF], fp32, tag="aa", name="pf2")
                for g in range(KS // 2):
                    nc.tensor.matmul(
                        pf2[:, :QF], wf2[:, g, :, m * 128:(m + 1) * 128],
                        hful[:, 1, g, :, :], start=(g == 0), stop=(g == KS // 2 - 1),
                        perf_mode=mybir.MatmulPerfMode.DoubleRow)
            y2T = ytp.tile([128, QF], bf16, tag="yT")
            nc.vector.tensor_scalar(
                y2T[:], pf2[:, :QF], 1.0 / W2SCALE, bias[:, 21 + m:22 + m],
                op0=ALU.mult, op1=ALU.add)
            if m < CS - 1:
                transpose_add(
                    y2T,
                    x2[:, t0:t0 + NB, m * 128:(m + 1) * 128],
                    x2[:, t0:t0 + NB, m * 128:(m + 1) * 128])
            else:
                ptr = psa.tile([128, NF], bf16, tag="aa", name="ptrC")
                for b in range(NB):
                    nc.tensor.transpose(
                        ptr[:, b * 128:(b + 1) * 128],
                        y2T[:, b * 128:(b + 1) * 128], ident[:])
                for b in range(NB):
                    t = t0 + b
                    nc.vector.tensor_tensor(
                        x2[:, t, m * 128:(m + 1) * 128],
                        ptr[:, b * 128:(b + 1) * 128],
                        x2[:, t, m * 128:(m + 1) * 128], ALU.add)
                    nc.sync.dma_start(yout_t[:, t, :], x2[:, t, :])
    return nc


def prep_inputs(x, w_qkv, b_qkv, w_proj, b_proj, w_fc1, b_fc1, w_fc2, b_fc2,
                g1, beta1, g2, beta2, n_cores=8):
    """Host-side preprocessing: fold LN affine + attention scale into
    weights/biases, cast to bf16/fp8, reshape to SBUF layouts, permute x."""
    scale_q = DH ** -0.5

    wq = (g1[:, None] * w_qkv[:, :C]) * scale_q
    wk = g1[:, None] * w_qkv[:, C:2 * C]
    wv_ = g1[:, None] * w_qkv[:, 2 * C:]
    bq = (b_qkv[:C] + beta1 @ w_qkv[:, :C]) * scale_q
    bk = b_qkv[C:2 * C] + beta1 @ w_qkv[:, C:2 * C]
    bv_ = b_qkv[2 * C:] + beta1 @ w_qkv[:, 2 * C:]
    # softmax rows sum to 1, so the V bias passes through attention
    # unchanged and folds into the projection bias: A(V + 1 bv^T) Wp + b
    # = (A V) Wp + (bv Wp + b).
    bp_ = b_proj + bv_ @ w_proj
    wf1_ = g2[:, None] * w_fc1
    bf1_ = b_fc1 + beta2 @ w_fc1

    def kx(w):
        n = w.shape[0] // 128
        return np.ascontiguousarray(
            w.reshape(n, 128, w.shape[1]).transpose(1, 0, 2)
        ).astype(ml_dtypes.bfloat16)

    wqk_l = kx(np.concatenate([wq, wk], axis=1))
    wv_l = kx(wv_)
    wp_l = kx(w_proj)
    wf1_l = kx(wf1_)
    # fc2 fp8 DoubleRow pair layout: wf2_dr[p, g, i, c] = w_fc2[(2g+i)*128+p, c]
    wf2_l = np.ascontiguousarray(
        (w_fc2 * W2SCALE).reshape(KS // 2, 2, 128, C).transpose(2, 0, 1, 3)
    ).astype(ml_dtypes.float8_e4m3)

    bias_h = np.zeros((128, NBIAS), np.float32)
    bias_h[:, 0:3] = bq.reshape(3, 128).T
    bias_h[:, 3:6] = bk.reshape(3, 128).T
    bias_h[:, 6:9] = bp_.reshape(3, 128).T
    bias_h[:, 9:21] = bf1_.reshape(12, 128).T
    bias_h[:, 21:24] = b_fc2.reshape(3, 128).T

    B, N, _ = x.shape
    half = N // 2
    in_maps = []
    for core in range(n_cores):
        b, hf = core // 2, core % 2
        own = x[b, hf * half:(hf + 1) * half]
        other = x[b, (1 - hf) * half:(2 - hf) * half]
        xin_core = np.ascontiguousarray(
            np.concatenate([own, other], axis=0), dtype=np.float32)
        in_maps.append({
            "xin": xin_core, "wqk": wqk_l, "wv": wv_l, "wp": wp_l,
            "wf1": wf1_l, "wf2": wf2_l, "bias": bias_h,
        })
    return in_maps


def assemble_output(results, B, N):
    half = N // 2
    y = np.empty((B, N, C), np.float32)
    for core, r in enumerate(results):
        b, hf = core // 2, core % 2
        y[b, hf * half:(hf + 1) * half] = r["yout"]
    return y


_CACHED = {}


def _get_compiled(SEQ):
    if SEQ not in _CACHED:
        from concourse import bacc
        nc = bacc.Bacc("TRN2", target_bir_lowering=False, debug=False)
        build(nc, SEQ=SEQ)
        nc.compile()
        _CACHED[SEQ] = nc
    return _CACHED[SEQ]


def kernel(x, w_qkv, b_qkv, w_proj, b_proj, w_fc1, b_fc1, w_fc2, b_fc2,
           g1, beta1, g2, beta2):
    from concourse.bass_utils import run_bass_kernel_spmd

    x = np.asarray(x, dtype=np.float32)
    B, N, _ = x.shape
    nc = _get_compiled(N)
    in_maps = prep_inputs(
        x, np.asarray(w_qkv, np.float32), np.asarray(b_qkv, np.float32),
        np.asarray(w_proj, np.float32), np.asarray(b_proj, np.float32),
        np.asarray(w_fc1, np.float32), np.asarray(b_fc1, np.float32),
        np.asarray(w_fc2, np.float32), np.asarray(b_fc2, np.float32),
        np.asarray(g1, np.float32), np.asarray(beta1, np.float32),
        np.asarray(g2, np.float32), np.asarray(beta2, np.float32),
        n_cores=2 * B)
    res = run_bass_kernel_spmd(
        nc, in_maps, core_ids=list(range(2 * B)), trace=False)
    return assemble_output(res.results, B=B, N=N)
